# revision 55
# baseline (speedup 1.0000x reference)
"""Trainium2 kernel for DWTFeatureModel.

Model: 3-level db4 DWT along time (256 -> 276 coeffs, reflect padding) for
each of B*64 channels, then a Conv3d whose kernel spans the whole
(276, 8, 8) volume (== full contraction to 64 features), bias, LeakyReLU.

The DWT is linear, so dwt(sig) = sig @ M for a fixed (256, 276) analysis
matrix M built from the db4 filter bank. The whole model then collapses to

    out[b, f] = leaky(sum_{s,hw} x[b, s, hw] * Weff[s, hw, f] + bias[f])
    Weff[s, hw, f] = sum_t M[s, t] * W[f, t, hw]

Pure batch-data-parallel over the 8 cores (256 batches each). The default
variant (RAW + HOST_FOLD) folds M into the conv weight on the host
(standard weight preprocessing, ~0.3 GFLOP once, exact fp64) and the
device runs the 2.1 GFLOP data contraction, hand-synchronized:

  sync:   HBM stream on one HWDGE ring: Weff half 0 (1 MB), x tiles
          (8 MB as contiguous 0.25-1 MB blocks, tapered at both ends),
          Weff half 1, bias; per-transfer semaphores.
  tensor: a few warmup matmuls (HAM clock-gate), then out^T += Weff_g^T
          @ x_g^T over 128 contraction chunks (K=128, M=64, N=256 bf16,
          fp32 PSUM accumulate), each gated only on its x tile's DMA.
  vector: + bias, LeakyReLU via max(y, 0.02*y); sync DMAs out^T back.

Alternate variants kept for reference: RAW without HOST_FOLD computes the
fold on the tensor engine (48 N=512 bf16 matmuls, PSUM-wave pipelined);
RAW=False is the TileContext version (auto-scheduled, ~9us slower in
framework barrier overhead).

The full-precision path runs everything except the bf16 rounding of x and
Weff in fp32; measured end-to-end absmax error vs the fp32 reference is
~2.5e-3 of the output scale (x's bf16 rounding dominates).

Host side shards/permutes/casts inputs per core and transposes the
(64, 256) per-core outputs back into the (2048, 64) result.
"""

from contextlib import ExitStack

import numpy as np

import concourse.bass as bass
from concourse import mybir
from concourse.bass_utils import run_bass_kernel_spmd

# pywt db4 analysis filters (identical constants to the model definition)
DEC_LO = [-0.010597401784997278, 0.032883011666982945, 0.030841381835986965,
          -0.18703481171888114, -0.02798376941698385, 0.6308807679295904,
          0.7148465705525415, 0.23037781330885523]
DEC_HI = [-0.23037781330885523, 0.7148465705525415, -0.6308807679295904,
          -0.02798376941698385, 0.18703481171888114, 0.030841381835986965,
          -0.032883011666982945, -0.010597401784997278]

B, T, F, TDWT = 2048, 256, 64, 276
J, L = 3, 8
NEG_SLOPE = 0.02
NCORES = 8
BC = B // NCORES          # 256 batches per core
G = 128                   # contraction chunks of 128 (= 2 s-blocks x 64 hw)
# x tile sizes in chunks: big tiles first, tapered at the end so the PE
# isn't left with a long chase after the last DMA byte lands
XTILES = [8, 8, 16, 16, 16, 16, 16, 16, 8, 4, 4]
NT = len(XTILES)
NWARM = 22                # PE warmup matmuls (bridge until the first x tile lands)
TCH = [(0, 128), (128, 128), (256, 20)]  # t-chunks of the 276 DWT coeffs


def _build_dwt_matrix():
    """M (T, TDWT) with dwt(sig) = sig @ M, matching the reference's
    multi-level reflect-padded strided cross-correlation."""
    h_lo = np.array(DEC_LO, np.float64)[::-1]
    h_hi = np.array(DEC_HI, np.float64)[::-1]
    lo = np.eye(T, dtype=np.float64)
    his = []
    for _ in range(J):
        n = lo.shape[-1]
        outsize = (n + L - 1) // 2
        p = 2 * (outsize - 1) - n + L
        xp = np.pad(lo, ((0, 0), (p // 2, (p + 1) // 2)), mode="reflect")
        idx = np.arange(outsize)[:, None] * 2 + np.arange(L)[None, :]
        win = xp[:, idx]
        his.append(win @ h_hi)
        lo = win @ h_lo
    return np.concatenate([lo] + his, axis=-1)  # (256, 276)


def _emit_raw(nc, xt, wt, bi, outT):
    """Hand-synchronized variant: no TileContext startup barrier or exit
    butterfly (saves ~15us of fixed overhead)."""
    f32 = mybir.dt.float32
    bf16 = mybir.dt.bfloat16
    CW = T + 64 * F

    # SBUF layout (static)
    cw_sb = [nc.alloc_sbuf_tensor(f"cw{ti}", [tsz, CW], bf16).ap()
             for ti, (t0, tsz) in enumerate(TCH)]
    weff = nc.alloc_sbuf_tensor("weff", [128, 2 * 64 * F], bf16).ap()
    xt_sb = [nc.alloc_sbuf_tensor(f"xs{t}", [128, xg, BC], bf16).ap()
             for t, xg in enumerate(XTILES)]
    wsrc = nc.alloc_sbuf_tensor("wsrc", [128, BC], bf16).ap()
    bias = nc.alloc_sbuf_tensor("bias", [F, 1], f32).ap()
    t1 = nc.alloc_sbuf_tensor("t1", [F, BC], f32).ap()
    y = nc.alloc_sbuf_tensor("y", [F, BC], f32).ap()

    # fold waves of 2 groups, triple-buffered over 6 PSUM banks (+1 acc)
    NWAVE = 8
    WGRP = 2

    with ExitStack() as es:
        fold_ps = [es.enter_context(
            nc.psum_tensor(f"fps{i}", [128, WGRP * 512], f32)).ap()
            for i in range(3)]
        acc = es.enter_context(nc.psum_tensor("accps", [F, BC], f32)).ap()
        cw_sems = [es.enter_context(nc.semaphore(f"cw{i}_sem"))
                   for i in range(len(TCH))]
        x_sems = [es.enter_context(nc.semaphore(f"x{t}_sem"))
                  for t in range(len(XTILES))]
        bias_sem = es.enter_context(nc.semaphore("bias_sem"))
        out_sem = es.enter_context(nc.semaphore("out_sem"))
        ws_sem = es.enter_context(nc.semaphore("ws_sem"))
        fold_sem = es.enter_context(nc.semaphore("fold_sem"))
        cast_sem = es.enter_context(nc.semaphore("cast_sem"))
        acc_sem = es.enter_context(nc.semaphore("acc_sem"))
        epi_sem = es.enter_context(nc.semaphore("epi_sem"))
        y_sem = es.enter_context(nc.semaphore("y_sem"))
        block = es.enter_context(nc.Block())

        def pwk(w, k):  # psum slice for group k of wave w
            return fold_ps[w % 3][:, k * 512:(k + 1) * 512]

        @block.sync
        def _(sync):
            for ti in (2, 0, 1):  # small tail chunk first
                t0, tsz = TCH[ti]
                sync.dma_start(cw_sb[ti][:], wt[t0:t0 + tsz, :]).then_inc(
                    cw_sems[ti], 16)
            off = 0
            for t, xg in enumerate(XTILES):
                foff = 128 * BC * off
                src = xt[foff: foff + 128 * xg * BC].rearrange(
                    "(p c b) -> p c b", p=128, c=xg)
                sync.dma_start(xt_sb[t][:], src).then_inc(x_sems[t], 16)
                off += xg
            sync.dma_start(bias[:], bi[:]).then_inc(bias_sem, 16)
            sync.wait_ge(y_sem, 1)
            sync.dma_start(outT[:], y[:]).then_inc(out_sem, 16)
            sync.wait_ge(out_sem, 16)

        @block.gpsimd
        def _(gpsimd):
            gpsimd.memset(wsrc[:], 0.0).then_inc(ws_sem, 1)

        @block.tensor
        def _(tensor):
            tensor.wait_ge(ws_sem, 1)
            for _ in range(NWARM):
                tensor.matmul(acc[0:1, :], wsrc[:, 0:1], wsrc[:],
                              start=True, stop=True)
            # fold
            for w in range(NWAVE):
                if w >= 3:
                    tensor.wait_ge(cast_sem, WGRP * (w - 2))
                for ti, (t0, tsz) in enumerate(TCH):
                    if w == 0:
                        tensor.wait_ge(cw_sems[ti], 16)
                    for k in range(WGRP):
                        grp = w * WGRP + k
                        sblk, hwg = grp // 8, grp % 8
                        mm = tensor.matmul(
                            pwk(w, k),
                            cw_sb[ti][:, sblk * 128:(sblk + 1) * 128],
                            cw_sb[ti][:, T + hwg * 512: T + (hwg + 1) * 512],
                            start=(ti == 0), stop=(ti == 2),
                        )
                mm.then_inc(fold_sem, 1)
            # main
            off = 0
            last_cast_wait = 0
            for t, xg in enumerate(XTILES):
                tensor.wait_ge(x_sems[t], 16)
                for c in range(xg):
                    g = off + c
                    sblk, hw = g // 64, g % 64
                    grp = sblk * 8 + hw // 8
                    if grp + 1 > last_cast_wait:
                        last_cast_wait = grp + 1
                        tensor.wait_ge(cast_sem, last_cast_wait)
                    mm = tensor.matmul(
                        acc[:],
                        weff[:, sblk * 4096 + hw * 64: sblk * 4096 + (hw + 1) * 64],
                        xt_sb[t][:, c, :],
                        start=(g == 0), stop=(g == G - 1),
                    )
                off += xg
            mm.then_inc(acc_sem, 1)

        @block.vector
        def _(vector):
            for w in range(NWAVE):
                vector.wait_ge(fold_sem, w + 1)
                for k in range(WGRP):
                    grp = w * WGRP + k
                    sblk, hwg = grp // 8, grp % 8
                    dst = weff[:, sblk * 4096 + hwg * 512:
                               sblk * 4096 + (hwg + 1) * 512]
                    vector.tensor_copy(dst, pwk(w, k)).then_inc(cast_sem, 1)
            vector.wait_ge(acc_sem, 1)
            vector.wait_ge(bias_sem, 16)
            vector.tensor_scalar_add(t1[:], acc[:], bias[:]).then_inc(epi_sem, 1)
            vector.wait_ge(epi_sem, 1)
            vector.scalar_tensor_tensor(
                y[:], t1[:], NEG_SLOPE, t1[:],
                op0=mybir.AluOpType.mult, op1=mybir.AluOpType.max,
            ).then_inc(y_sem, 1)


def _emit_raw_hostfold(nc, xt, wf, bi, outT):
    """Host-folded variant: Weff arrives precomputed (weight preprocessing);
    the device runs the 2.1 GFLOP data contraction + epilogue."""
    f32 = mybir.dt.float32
    bf16 = mybir.dt.bfloat16

    weff = nc.alloc_sbuf_tensor("weff", [128, 2 * 64 * F], bf16).ap()
    xt_sb = [nc.alloc_sbuf_tensor(f"xs{t}", [128, xg, BC], bf16).ap()
             for t, xg in enumerate(XTILES)]
    wsrc = nc.alloc_sbuf_tensor("wsrc", [128, BC], bf16).ap()
    bias = nc.alloc_sbuf_tensor("bias", [F, 1], f32).ap()
    t1 = nc.alloc_sbuf_tensor("t1", [F, BC], f32).ap()
    y = nc.alloc_sbuf_tensor("y", [F, BC], f32).ap()

    with ExitStack() as es:
        acc = es.enter_context(nc.psum_tensor("accps", [F, BC], f32)).ap()
        wf_sems = [es.enter_context(nc.semaphore(f"wf{i}_sem")) for i in range(2)]
        x_sems = [es.enter_context(nc.semaphore(f"x{t}_sem"))
                  for t in range(len(XTILES))]
        bias_sem = es.enter_context(nc.semaphore("bias_sem"))
        out_sem = es.enter_context(nc.semaphore("out_sem"))
        ws_sem = es.enter_context(nc.semaphore("ws_sem"))
        acc_sem = es.enter_context(nc.semaphore("acc_sem"))
        epi_sem = es.enter_context(nc.semaphore("epi_sem"))
        y_sem = es.enter_context(nc.semaphore("y_sem"))
        block = es.enter_context(nc.Block())

        @block.sync
        def _(sync):
            # weff half 0 (covers the first 64 chunks), first two x tiles,
            # then weff half 1, then the x stream
            sync.dma_start(weff[:, 0:4096], wf[:, 0:4096]).then_inc(wf_sems[0], 16)
            offs = np.cumsum([0] + XTILES)

            def xdma(t):
                foff = 128 * BC * int(offs[t])
                src = xt[foff: foff + 128 * XTILES[t] * BC].rearrange(
                    "(p c b) -> p c b", p=128, c=XTILES[t])
                sync.dma_start(xt_sb[t][:], src).then_inc(x_sems[t], 16)

            xdma(0)
            xdma(1)
            sync.dma_start(weff[:, 4096:], wf[:, 4096:]).then_inc(wf_sems[1], 16)
            for t in range(2, len(XTILES)):
                xdma(t)
            sync.dma_start(bias[:], bi[:]).then_inc(bias_sem, 16)
            sync.wait_ge(y_sem, 1)
            sync.dma_start(outT[:], y[:]).then_inc(out_sem, 16)
            sync.wait_ge(out_sem, 16)

        @block.gpsimd
        def _(gpsimd):
            gpsimd.memset(wsrc[:], 0.0).then_inc(ws_sem, 1)

        @block.tensor
        def _(tensor):
            tensor.wait_ge(ws_sem, 1)
            for _ in range(NWARM):
                tensor.matmul(acc[0:1, :], wsrc[:, 0:1], wsrc[:],
                              start=True, stop=True)
            off = 0
            waited_wf1 = False
            for t, xg in enumerate(XTILES):
                tensor.wait_ge(x_sems[t], 16)
                if t == 0:
                    tensor.wait_ge(wf_sems[0], 16)
                for c in range(xg):
                    g = off + c
                    sblk, hw = g // 64, g % 64
                    if sblk == 1 and not waited_wf1:
                        waited_wf1 = True
                        tensor.wait_ge(wf_sems[1], 16)
                    mm = tensor.matmul(
                        acc[:],
                        weff[:, sblk * 4096 + hw * 64: sblk * 4096 + (hw + 1) * 64],
                        xt_sb[t][:, c, :],
                        start=(g == 0), stop=(g == G - 1),
                    )
                off += xg
            mm.then_inc(acc_sem, 1)

        @block.vector
        def _(vector):
            vector.wait_ge(acc_sem, 1)
            vector.wait_ge(bias_sem, 16)
            vector.tensor_scalar_add(t1[:], acc[:], bias[:]).then_inc(epi_sem, 1)
            vector.wait_ge(epi_sem, 1)
            vector.scalar_tensor_tensor(
                y[:], t1[:], NEG_SLOPE, t1[:],
                op0=mybir.AluOpType.mult, op1=mybir.AluOpType.max,
            ).then_inc(y_sem, 1)


_CACHE = {}


HOST_FOLD = True


def _get_kernel():
    if "nc" not in _CACHE:
        f32 = mybir.dt.float32
        bf16 = mybir.dt.bfloat16
        nc = bass.Bass("TRN2", target_bir_lowering=False, debug=False)
        xt_d = nc.dram_tensor("xt", [G * 128 * BC], bf16, kind="ExternalInput")
        bi_d = nc.dram_tensor("bi", [F, 1], f32, kind="ExternalInput")
        out_d = nc.dram_tensor("outT", [F, BC], f32, kind="ExternalOutput")
        if HOST_FOLD:
            wf_d = nc.dram_tensor("wf", [128, 2 * 64 * F], bf16,
                                  kind="ExternalInput")
            _emit_raw_hostfold(nc, xt_d.ap(), wf_d.ap(), bi_d.ap(), out_d.ap())
        else:
            wt_d = nc.dram_tensor("wt", [TDWT, T + 64 * F], bf16,
                                  kind="ExternalInput")
            _emit_raw(nc, xt_d.ap(), wt_d.ap(), bi_d.ap(), out_d.ap())
        _CACHE["nc"] = nc
    return _CACHE["nc"]


def make_in_maps(x, W, b):
    import ml_dtypes
    bf16 = ml_dtypes.bfloat16
    dwt_m = _build_dwt_matrix()
    bi = np.ascontiguousarray(b.reshape(F, 1)).astype(np.float32)
    if HOST_FOLD:
        # weight preprocessing: fold the DWT matrix into the conv weight
        A = W[:, 0].reshape(F, TDWT, 64).transpose(1, 2, 0).reshape(TDWT, -1)
        weff = (dwt_m @ A.astype(np.float64)).reshape(T, 64, F)    # (s, hw, f)
        wf = np.ascontiguousarray(
            weff.reshape(2, 128, 64 * F).transpose(1, 0, 2)
        ).reshape(128, 2 * 64 * F).astype(bf16)
        wblob = {"wf": wf}
    else:
        dm = dwt_m.T                                               # (276, 256)
        wtc = W[:, 0].reshape(F, TDWT, 64).transpose(1, 2, 0).reshape(TDWT, -1)
        wt = np.ascontiguousarray(
            np.concatenate([dm, wtc], axis=1)).astype(bf16)        # (276, 4352)
        wblob = {"wt": wt}
    in_maps = []
    for c in range(NCORES):
        # chunk g = sblk*64 + hw holds rows [s_in, b]; tiles of XTILES[t]
        # chunks are stored back-to-back as [p, chunk, b] blocks so each
        # tile is one contiguous DMA.
        xc = x[c * BC:(c + 1) * BC, 0].astype(bf16)                # (BC, 256, 8, 8)
        xg = xc.reshape(BC, 2, 128, 64).transpose(1, 3, 2, 0)      # (sblk, hw, s_in, b)
        xg = xg.reshape(G, 128, BC)                                # (g, p, b)
        parts, off = [], 0
        for n in XTILES:
            parts.append(np.ascontiguousarray(
                xg[off:off + n].transpose(1, 0, 2)).reshape(-1))   # (p, c, b) flat
            off += n
        in_maps.append({"xt": np.concatenate(parts), "bi": bi, **wblob})
    return in_maps


def kernel(x, W, b, _trace=False):
    nc = _get_kernel()
    in_maps = make_in_maps(np.asarray(x), np.asarray(W), np.asarray(b))
    res = run_bass_kernel_spmd(nc, in_maps, list(range(NCORES)), trace=_trace)
    out = np.empty((B, F), np.float32)
    for c in range(NCORES):
        out[c * BC:(c + 1) * BC] = res.results[c]["outT"].T
    if _trace:
        return out, res
    return out


# revision 56
# speedup vs baseline: 1.0567x; 1.0567x over previous
"""Trainium2 kernel for DWTFeatureModel.

Model: 3-level db4 DWT along time (256 -> 276 coeffs, reflect padding) for
each of B*64 channels, then a Conv3d whose kernel spans the whole
(276, 8, 8) volume (== full contraction to 64 features), bias, LeakyReLU.

The DWT is linear, so dwt(sig) = sig @ M for a fixed (256, 276) analysis
matrix M built from the db4 filter bank. The whole model then collapses to

    out[b, f] = leaky(sum_{s,hw} x[b, s, hw] * Weff[s, hw, f] + bias[f])
    Weff[s, hw, f] = sum_t M[s, t] * W[f, t, hw]

Pure batch-data-parallel over the 8 cores (256 batches each). The default
variant (RAW + HOST_FOLD) folds M into the conv weight on the host
(standard weight preprocessing, ~0.3 GFLOP once, exact fp64) and the
device runs the 2.1 GFLOP data contraction, hand-synchronized:

  sync:   HBM stream on one HWDGE ring: Weff half 0 (1 MB), x tiles
          (8 MB as contiguous 0.25-1 MB blocks, tapered at both ends),
          Weff half 1, bias; per-transfer semaphores.
  tensor: a few warmup matmuls (HAM clock-gate), then out^T += Weff_g^T
          @ x_g^T over 128 contraction chunks (K=128, M=64, N=256 bf16,
          fp32 PSUM accumulate), each gated only on its x tile's DMA.
  vector: + bias, LeakyReLU via max(y, 0.02*y); sync DMAs out^T back.

Alternate variants kept for reference: RAW without HOST_FOLD computes the
fold on the tensor engine (48 N=512 bf16 matmuls, PSUM-wave pipelined);
RAW=False is the TileContext version (auto-scheduled, ~9us slower in
framework barrier overhead).

The full-precision path runs everything except the bf16 rounding of x and
Weff in fp32; measured end-to-end absmax error vs the fp32 reference is
~2.5e-3 of the output scale (x's bf16 rounding dominates).

Host side shards/permutes/casts inputs per core and transposes the
(64, 256) per-core outputs back into the (2048, 64) result.
"""

from contextlib import ExitStack

import numpy as np

import concourse.bass as bass
from concourse import mybir
from concourse.bass_utils import run_bass_kernel_spmd

# pywt db4 analysis filters (identical constants to the model definition)
DEC_LO = [-0.010597401784997278, 0.032883011666982945, 0.030841381835986965,
          -0.18703481171888114, -0.02798376941698385, 0.6308807679295904,
          0.7148465705525415, 0.23037781330885523]
DEC_HI = [-0.23037781330885523, 0.7148465705525415, -0.6308807679295904,
          -0.02798376941698385, 0.18703481171888114, 0.030841381835986965,
          -0.032883011666982945, -0.010597401784997278]

B, T, F, TDWT = 2048, 256, 64, 276
J, L = 3, 8
NEG_SLOPE = 0.02
NCORES = 8
BC = B // NCORES          # 256 batches per core
G = 128                   # contraction chunks of 128 (= 2 s-blocks x 64 hw)
# x tile sizes in chunks: big tiles first, tapered at the end so the PE
# isn't left with a long chase after the last DMA byte lands
XTILES = [8, 8, 16, 16, 16, 16, 16, 16, 8, 4, 4]
NT = len(XTILES)
NWARM = 22                # PE warmup matmuls (bridge until the first x tile lands)
TCH = [(0, 128), (128, 128), (256, 20)]  # t-chunks of the 276 DWT coeffs


def _build_dwt_matrix():
    """M (T, TDWT) with dwt(sig) = sig @ M, matching the reference's
    multi-level reflect-padded strided cross-correlation."""
    h_lo = np.array(DEC_LO, np.float64)[::-1]
    h_hi = np.array(DEC_HI, np.float64)[::-1]
    lo = np.eye(T, dtype=np.float64)
    his = []
    for _ in range(J):
        n = lo.shape[-1]
        outsize = (n + L - 1) // 2
        p = 2 * (outsize - 1) - n + L
        xp = np.pad(lo, ((0, 0), (p // 2, (p + 1) // 2)), mode="reflect")
        idx = np.arange(outsize)[:, None] * 2 + np.arange(L)[None, :]
        win = xp[:, idx]
        his.append(win @ h_hi)
        lo = win @ h_lo
    return np.concatenate([lo] + his, axis=-1)  # (256, 276)


def _emit_raw(nc, xt, wt, bi, outT):
    """Hand-synchronized variant: no TileContext startup barrier or exit
    butterfly (saves ~15us of fixed overhead)."""
    f32 = mybir.dt.float32
    bf16 = mybir.dt.bfloat16
    CW = T + 64 * F

    # SBUF layout (static)
    cw_sb = [nc.alloc_sbuf_tensor(f"cw{ti}", [tsz, CW], bf16).ap()
             for ti, (t0, tsz) in enumerate(TCH)]
    weff = nc.alloc_sbuf_tensor("weff", [128, 2 * 64 * F], bf16).ap()
    xt_sb = [nc.alloc_sbuf_tensor(f"xs{t}", [128, xg, BC], bf16).ap()
             for t, xg in enumerate(XTILES)]
    wsrc = nc.alloc_sbuf_tensor("wsrc", [128, BC], bf16).ap()
    bias = nc.alloc_sbuf_tensor("bias", [F, 1], f32).ap()
    t1 = nc.alloc_sbuf_tensor("t1", [F, BC], f32).ap()
    y = nc.alloc_sbuf_tensor("y", [F, BC], f32).ap()

    # fold waves of 2 groups, triple-buffered over 6 PSUM banks (+1 acc)
    NWAVE = 8
    WGRP = 2

    with ExitStack() as es:
        fold_ps = [es.enter_context(
            nc.psum_tensor(f"fps{i}", [128, WGRP * 512], f32)).ap()
            for i in range(3)]
        acc = es.enter_context(nc.psum_tensor("accps", [F, BC], f32)).ap()
        cw_sems = [es.enter_context(nc.semaphore(f"cw{i}_sem"))
                   for i in range(len(TCH))]
        x_sems = [es.enter_context(nc.semaphore(f"x{t}_sem"))
                  for t in range(len(XTILES))]
        bias_sem = es.enter_context(nc.semaphore("bias_sem"))
        out_sem = es.enter_context(nc.semaphore("out_sem"))
        ws_sem = es.enter_context(nc.semaphore("ws_sem"))
        fold_sem = es.enter_context(nc.semaphore("fold_sem"))
        cast_sem = es.enter_context(nc.semaphore("cast_sem"))
        acc_sem = es.enter_context(nc.semaphore("acc_sem"))
        epi_sem = es.enter_context(nc.semaphore("epi_sem"))
        y_sem = es.enter_context(nc.semaphore("y_sem"))
        block = es.enter_context(nc.Block())

        def pwk(w, k):  # psum slice for group k of wave w
            return fold_ps[w % 3][:, k * 512:(k + 1) * 512]

        @block.sync
        def _(sync):
            for ti in (2, 0, 1):  # small tail chunk first
                t0, tsz = TCH[ti]
                sync.dma_start(cw_sb[ti][:], wt[t0:t0 + tsz, :]).then_inc(
                    cw_sems[ti], 16)
            off = 0
            for t, xg in enumerate(XTILES):
                foff = 128 * BC * off
                src = xt[foff: foff + 128 * xg * BC].rearrange(
                    "(p c b) -> p c b", p=128, c=xg)
                sync.dma_start(xt_sb[t][:], src).then_inc(x_sems[t], 16)
                off += xg
            sync.dma_start(bias[:], bi[:]).then_inc(bias_sem, 16)
            sync.wait_ge(y_sem, 1)
            sync.dma_start(outT[:], y[:]).then_inc(out_sem, 16)
            sync.wait_ge(out_sem, 16)

        @block.gpsimd
        def _(gpsimd):
            gpsimd.memset(wsrc[:], 0.0).then_inc(ws_sem, 1)

        @block.tensor
        def _(tensor):
            tensor.wait_ge(ws_sem, 1)
            for _ in range(NWARM):
                tensor.matmul(acc[0:1, :], wsrc[:, 0:1], wsrc[:],
                              start=True, stop=True)
            # fold
            for w in range(NWAVE):
                if w >= 3:
                    tensor.wait_ge(cast_sem, WGRP * (w - 2))
                for ti, (t0, tsz) in enumerate(TCH):
                    if w == 0:
                        tensor.wait_ge(cw_sems[ti], 16)
                    for k in range(WGRP):
                        grp = w * WGRP + k
                        sblk, hwg = grp // 8, grp % 8
                        mm = tensor.matmul(
                            pwk(w, k),
                            cw_sb[ti][:, sblk * 128:(sblk + 1) * 128],
                            cw_sb[ti][:, T + hwg * 512: T + (hwg + 1) * 512],
                            start=(ti == 0), stop=(ti == 2),
                        )
                mm.then_inc(fold_sem, 1)
            # main
            off = 0
            last_cast_wait = 0
            for t, xg in enumerate(XTILES):
                tensor.wait_ge(x_sems[t], 16)
                for c in range(xg):
                    g = off + c
                    sblk, hw = g // 64, g % 64
                    grp = sblk * 8 + hw // 8
                    if grp + 1 > last_cast_wait:
                        last_cast_wait = grp + 1
                        tensor.wait_ge(cast_sem, last_cast_wait)
                    mm = tensor.matmul(
                        acc[:],
                        weff[:, sblk * 4096 + hw * 64: sblk * 4096 + (hw + 1) * 64],
                        xt_sb[t][:, c, :],
                        start=(g == 0), stop=(g == G - 1),
                    )
                off += xg
            mm.then_inc(acc_sem, 1)

        @block.vector
        def _(vector):
            for w in range(NWAVE):
                vector.wait_ge(fold_sem, w + 1)
                for k in range(WGRP):
                    grp = w * WGRP + k
                    sblk, hwg = grp // 8, grp % 8
                    dst = weff[:, sblk * 4096 + hwg * 512:
                               sblk * 4096 + (hwg + 1) * 512]
                    vector.tensor_copy(dst, pwk(w, k)).then_inc(cast_sem, 1)
            vector.wait_ge(acc_sem, 1)
            vector.wait_ge(bias_sem, 16)
            vector.tensor_scalar_add(t1[:], acc[:], bias[:]).then_inc(epi_sem, 1)
            vector.wait_ge(epi_sem, 1)
            vector.scalar_tensor_tensor(
                y[:], t1[:], NEG_SLOPE, t1[:],
                op0=mybir.AluOpType.mult, op1=mybir.AluOpType.max,
            ).then_inc(y_sem, 1)


def _emit_raw_hostfold(nc, xt, wf, bi, outT):
    """Host-folded variant: Weff arrives precomputed (weight preprocessing);
    the device runs the 2.1 GFLOP data contraction + epilogue."""
    f32 = mybir.dt.float32
    bf16 = mybir.dt.bfloat16

    weff = nc.alloc_sbuf_tensor("weff", [128, 2 * 64 * F], bf16).ap()
    xt_sb = [nc.alloc_sbuf_tensor(f"xs{t}", [128, xg, BC], bf16).ap()
             for t, xg in enumerate(XTILES)]
    wsrc = nc.alloc_sbuf_tensor("wsrc", [128, BC], bf16).ap()
    bias = nc.alloc_sbuf_tensor("bias", [F, 1], f32).ap()
    t1 = nc.alloc_sbuf_tensor("t1", [F, BC], f32).ap()
    y = nc.alloc_sbuf_tensor("y", [F, BC], f32).ap()

    with ExitStack() as es:
        acc = es.enter_context(nc.psum_tensor("accps", [F, BC], f32)).ap()
        wf_sems = [es.enter_context(nc.semaphore(f"wf{i}_sem")) for i in range(2)]
        x_sems = [es.enter_context(nc.semaphore(f"x{t}_sem"))
                  for t in range(len(XTILES))]
        bias_sem = es.enter_context(nc.semaphore("bias_sem"))
        out_sem = es.enter_context(nc.semaphore("out_sem"))
        ws_sem = es.enter_context(nc.semaphore("ws_sem"))
        acc_sem = es.enter_context(nc.semaphore("acc_sem"))
        epi_sem = es.enter_context(nc.semaphore("epi_sem"))
        y_sem = es.enter_context(nc.semaphore("y_sem"))
        block = es.enter_context(nc.Block())

        @block.sync
        def _(sync):
            # weff half 0 (covers the first 64 chunks), first two x tiles,
            # then weff half 1, then the x stream
            sync.dma_start(weff[:, 0:4096], wf[:, 0:4096]).then_inc(wf_sems[0], 16)
            offs = np.cumsum([0] + XTILES)

            def xdma(t):
                foff = 128 * BC * int(offs[t])
                src = xt[foff: foff + 128 * XTILES[t] * BC].rearrange(
                    "(p c b) -> p c b", p=128, c=XTILES[t])
                sync.dma_start(xt_sb[t][:], src).then_inc(x_sems[t], 16)

            xdma(0)
            xdma(1)
            sync.dma_start(weff[:, 4096:], wf[:, 4096:]).then_inc(wf_sems[1], 16)
            for t in range(2, len(XTILES)):
                xdma(t)
            sync.wait_ge(y_sem, 1)
            sync.dma_start(outT[:], y[:]).then_inc(out_sem, 16)
            sync.wait_ge(out_sem, 16)

        @block.scalar
        def _(scalar):
            # tiny bias load on the otherwise idle second HWDGE ring, so it
            # never queues behind the 10 MB x stream
            scalar.dma_start(bias[:], bi[:]).then_inc(bias_sem, 16)

        @block.gpsimd
        def _(gpsimd):
            gpsimd.memset(wsrc[:], 0.0).then_inc(ws_sem, 1)

        @block.tensor
        def _(tensor):
            tensor.wait_ge(ws_sem, 1)
            for _ in range(NWARM):
                tensor.matmul(acc[0:1, :], wsrc[:, 0:1], wsrc[:],
                              start=True, stop=True)
            off = 0
            waited_wf1 = False
            for t, xg in enumerate(XTILES):
                tensor.wait_ge(x_sems[t], 16)
                if t == 0:
                    tensor.wait_ge(wf_sems[0], 16)
                for c in range(xg):
                    g = off + c
                    sblk, hw = g // 64, g % 64
                    if sblk == 1 and not waited_wf1:
                        waited_wf1 = True
                        tensor.wait_ge(wf_sems[1], 16)
                    mm = tensor.matmul(
                        acc[:],
                        weff[:, sblk * 4096 + hw * 64: sblk * 4096 + (hw + 1) * 64],
                        xt_sb[t][:, c, :],
                        start=(g == 0), stop=(g == G - 1),
                    )
                off += xg
            mm.then_inc(acc_sem, 1)

        @block.vector
        def _(vector):
            vector.wait_ge(acc_sem, 1)
            vector.wait_ge(bias_sem, 16)
            vector.tensor_scalar_add(t1[:], acc[:], bias[:]).then_inc(epi_sem, 1)
            vector.wait_ge(epi_sem, 1)
            vector.scalar_tensor_tensor(
                y[:], t1[:], NEG_SLOPE, t1[:],
                op0=mybir.AluOpType.mult, op1=mybir.AluOpType.max,
            ).then_inc(y_sem, 1)


_CACHE = {}


HOST_FOLD = True


def _get_kernel():
    if "nc" not in _CACHE:
        f32 = mybir.dt.float32
        bf16 = mybir.dt.bfloat16
        nc = bass.Bass("TRN2", target_bir_lowering=False, debug=False)
        xt_d = nc.dram_tensor("xt", [G * 128 * BC], bf16, kind="ExternalInput")
        bi_d = nc.dram_tensor("bi", [F, 1], f32, kind="ExternalInput")
        out_d = nc.dram_tensor("outT", [F, BC], f32, kind="ExternalOutput")
        if HOST_FOLD:
            wf_d = nc.dram_tensor("wf", [128, 2 * 64 * F], bf16,
                                  kind="ExternalInput")
            _emit_raw_hostfold(nc, xt_d.ap(), wf_d.ap(), bi_d.ap(), out_d.ap())
        else:
            wt_d = nc.dram_tensor("wt", [TDWT, T + 64 * F], bf16,
                                  kind="ExternalInput")
            _emit_raw(nc, xt_d.ap(), wt_d.ap(), bi_d.ap(), out_d.ap())
        _CACHE["nc"] = nc
    return _CACHE["nc"]


def make_in_maps(x, W, b):
    import ml_dtypes
    bf16 = ml_dtypes.bfloat16
    dwt_m = _build_dwt_matrix()
    bi = np.ascontiguousarray(b.reshape(F, 1)).astype(np.float32)
    if HOST_FOLD:
        # weight preprocessing: fold the DWT matrix into the conv weight
        A = W[:, 0].reshape(F, TDWT, 64).transpose(1, 2, 0).reshape(TDWT, -1)
        weff = (dwt_m @ A.astype(np.float64)).reshape(T, 64, F)    # (s, hw, f)
        wf = np.ascontiguousarray(
            weff.reshape(2, 128, 64 * F).transpose(1, 0, 2)
        ).reshape(128, 2 * 64 * F).astype(bf16)
        wblob = {"wf": wf}
    else:
        dm = dwt_m.T                                               # (276, 256)
        wtc = W[:, 0].reshape(F, TDWT, 64).transpose(1, 2, 0).reshape(TDWT, -1)
        wt = np.ascontiguousarray(
            np.concatenate([dm, wtc], axis=1)).astype(bf16)        # (276, 4352)
        wblob = {"wt": wt}
    in_maps = []
    for c in range(NCORES):
        # chunk g = sblk*64 + hw holds rows [s_in, b]; tiles of XTILES[t]
        # chunks are stored back-to-back as [p, chunk, b] blocks so each
        # tile is one contiguous DMA.
        xc = x[c * BC:(c + 1) * BC, 0].astype(bf16)                # (BC, 256, 8, 8)
        xg = xc.reshape(BC, 2, 128, 64).transpose(1, 3, 2, 0)      # (sblk, hw, s_in, b)
        xg = xg.reshape(G, 128, BC)                                # (g, p, b)
        parts, off = [], 0
        for n in XTILES:
            parts.append(np.ascontiguousarray(
                xg[off:off + n].transpose(1, 0, 2)).reshape(-1))   # (p, c, b) flat
            off += n
        in_maps.append({"xt": np.concatenate(parts), "bi": bi, **wblob})
    return in_maps


def kernel(x, W, b, _trace=False):
    nc = _get_kernel()
    in_maps = make_in_maps(np.asarray(x), np.asarray(W), np.asarray(b))
    res = run_bass_kernel_spmd(nc, in_maps, list(range(NCORES)), trace=_trace)
    out = np.empty((B, F), np.float32)
    for c in range(NCORES):
        out[c * BC:(c + 1) * BC] = res.results[c]["outT"].T
    if _trace:
        return out, res
    return out


# revision 57
# speedup vs baseline: 1.0773x; 1.0195x over previous
"""Trainium2 kernel for DWTFeatureModel.

Model: 3-level db4 DWT along time (256 -> 276 coeffs, reflect padding) for
each of B*64 channels, then a Conv3d whose kernel spans the whole
(276, 8, 8) volume (== full contraction to 64 features), bias, LeakyReLU.

The DWT is linear, so dwt(sig) = sig @ M for a fixed (256, 276) analysis
matrix M built from the db4 filter bank. The whole model then collapses to

    out[b, f] = leaky(sum_{s,hw} x[b, s, hw] * Weff[s, hw, f] + bias[f])
    Weff[s, hw, f] = sum_t M[s, t] * W[f, t, hw]

Pure batch-data-parallel over the 8 cores (256 batches each). The default
variant (RAW + HOST_FOLD) folds M into the conv weight on the host
(standard weight preprocessing, ~0.3 GFLOP once, exact fp64) and the
device runs the 2.1 GFLOP data contraction, hand-synchronized:

  sync:   HBM stream on one HWDGE ring: Weff half 0 (1 MB), x tiles
          (8 MB as contiguous 0.25-1 MB blocks, tapered at both ends),
          Weff half 1, bias; per-transfer semaphores.
  tensor: a few warmup matmuls (HAM clock-gate), then out^T += Weff_g^T
          @ x_g^T over 128 contraction chunks (K=128, M=64, N=256 bf16,
          fp32 PSUM accumulate), each gated only on its x tile's DMA.
  vector: + bias, LeakyReLU via max(y, 0.02*y); sync DMAs out^T back.

Alternate variants kept for reference: RAW without HOST_FOLD computes the
fold on the tensor engine (48 N=512 bf16 matmuls, PSUM-wave pipelined);
RAW=False is the TileContext version (auto-scheduled, ~9us slower in
framework barrier overhead).

The full-precision path runs everything except the bf16 rounding of x and
Weff in fp32; measured end-to-end absmax error vs the fp32 reference is
~2.5e-3 of the output scale (x's bf16 rounding dominates).

Host side shards/permutes/casts inputs per core and transposes the
(64, 256) per-core outputs back into the (2048, 64) result.
"""

from contextlib import ExitStack

import numpy as np

import concourse.bass as bass
from concourse import mybir
from concourse.bass_utils import run_bass_kernel_spmd

# pywt db4 analysis filters (identical constants to the model definition)
DEC_LO = [-0.010597401784997278, 0.032883011666982945, 0.030841381835986965,
          -0.18703481171888114, -0.02798376941698385, 0.6308807679295904,
          0.7148465705525415, 0.23037781330885523]
DEC_HI = [-0.23037781330885523, 0.7148465705525415, -0.6308807679295904,
          -0.02798376941698385, 0.18703481171888114, 0.030841381835986965,
          -0.032883011666982945, -0.010597401784997278]

B, T, F, TDWT = 2048, 256, 64, 276
J, L = 3, 8
NEG_SLOPE = 0.02
NCORES = 8
BC = B // NCORES          # 256 batches per core
G = 128                   # contraction chunks of 128 (= 2 s-blocks x 64 hw)
# x tile sizes in chunks: big tiles first, tapered at the end so the PE
# isn't left with a long chase after the last DMA byte lands
XTILES = [8, 8, 16, 16, 16, 16, 16, 16, 8, 4, 4]
NT = len(XTILES)
NWARM = 22                # PE warmup matmuls (bridge until the first x tile lands)
TCH = [(0, 128), (128, 128), (256, 20)]  # t-chunks of the 276 DWT coeffs


def _build_dwt_matrix():
    """M (T, TDWT) with dwt(sig) = sig @ M, matching the reference's
    multi-level reflect-padded strided cross-correlation."""
    h_lo = np.array(DEC_LO, np.float64)[::-1]
    h_hi = np.array(DEC_HI, np.float64)[::-1]
    lo = np.eye(T, dtype=np.float64)
    his = []
    for _ in range(J):
        n = lo.shape[-1]
        outsize = (n + L - 1) // 2
        p = 2 * (outsize - 1) - n + L
        xp = np.pad(lo, ((0, 0), (p // 2, (p + 1) // 2)), mode="reflect")
        idx = np.arange(outsize)[:, None] * 2 + np.arange(L)[None, :]
        win = xp[:, idx]
        his.append(win @ h_hi)
        lo = win @ h_lo
    return np.concatenate([lo] + his, axis=-1)  # (256, 276)


def _emit_raw(nc, xt, wt, bi, outT):
    """Hand-synchronized variant: no TileContext startup barrier or exit
    butterfly (saves ~15us of fixed overhead)."""
    f32 = mybir.dt.float32
    bf16 = mybir.dt.bfloat16
    CW = T + 64 * F

    # SBUF layout (static)
    cw_sb = [nc.alloc_sbuf_tensor(f"cw{ti}", [tsz, CW], bf16).ap()
             for ti, (t0, tsz) in enumerate(TCH)]
    weff = nc.alloc_sbuf_tensor("weff", [128, 2 * 64 * F], bf16).ap()
    xt_sb = [nc.alloc_sbuf_tensor(f"xs{t}", [128, xg, BC], bf16).ap()
             for t, xg in enumerate(XTILES)]
    wsrc = nc.alloc_sbuf_tensor("wsrc", [128, BC], bf16).ap()
    bias = nc.alloc_sbuf_tensor("bias", [F, 1], f32).ap()
    t1 = nc.alloc_sbuf_tensor("t1", [F, BC], f32).ap()
    y = nc.alloc_sbuf_tensor("y", [F, BC], f32).ap()

    # fold waves of 2 groups, triple-buffered over 6 PSUM banks (+1 acc)
    NWAVE = 8
    WGRP = 2

    with ExitStack() as es:
        fold_ps = [es.enter_context(
            nc.psum_tensor(f"fps{i}", [128, WGRP * 512], f32)).ap()
            for i in range(3)]
        acc = es.enter_context(nc.psum_tensor("accps", [F, BC], f32)).ap()
        cw_sems = [es.enter_context(nc.semaphore(f"cw{i}_sem"))
                   for i in range(len(TCH))]
        x_sems = [es.enter_context(nc.semaphore(f"x{t}_sem"))
                  for t in range(len(XTILES))]
        bias_sem = es.enter_context(nc.semaphore("bias_sem"))
        out_sem = es.enter_context(nc.semaphore("out_sem"))
        ws_sem = es.enter_context(nc.semaphore("ws_sem"))
        fold_sem = es.enter_context(nc.semaphore("fold_sem"))
        cast_sem = es.enter_context(nc.semaphore("cast_sem"))
        acc_sem = es.enter_context(nc.semaphore("acc_sem"))
        epi_sem = es.enter_context(nc.semaphore("epi_sem"))
        y_sem = es.enter_context(nc.semaphore("y_sem"))
        block = es.enter_context(nc.Block(no_gpsimd_drain=True))

        def pwk(w, k):  # psum slice for group k of wave w
            return fold_ps[w % 3][:, k * 512:(k + 1) * 512]

        @block.sync
        def _(sync):
            for ti in (2, 0, 1):  # small tail chunk first
                t0, tsz = TCH[ti]
                sync.dma_start(cw_sb[ti][:], wt[t0:t0 + tsz, :]).then_inc(
                    cw_sems[ti], 16)
            off = 0
            for t, xg in enumerate(XTILES):
                foff = 128 * BC * off
                src = xt[foff: foff + 128 * xg * BC].rearrange(
                    "(p c b) -> p c b", p=128, c=xg)
                sync.dma_start(xt_sb[t][:], src).then_inc(x_sems[t], 16)
                off += xg
            sync.dma_start(bias[:], bi[:]).then_inc(bias_sem, 16)
            sync.wait_ge(y_sem, 1)
            sync.dma_start(outT[:], y[:]).then_inc(out_sem, 16)
            sync.wait_ge(out_sem, 16)

        @block.gpsimd
        def _(gpsimd):
            gpsimd.memset(wsrc[:], 0.0).then_inc(ws_sem, 1)

        @block.tensor
        def _(tensor):
            tensor.wait_ge(ws_sem, 1)
            for _ in range(NWARM):
                tensor.matmul(acc[0:1, :], wsrc[:, 0:1], wsrc[:],
                              start=True, stop=True)
            # fold
            for w in range(NWAVE):
                if w >= 3:
                    tensor.wait_ge(cast_sem, WGRP * (w - 2))
                for ti, (t0, tsz) in enumerate(TCH):
                    if w == 0:
                        tensor.wait_ge(cw_sems[ti], 16)
                    for k in range(WGRP):
                        grp = w * WGRP + k
                        sblk, hwg = grp // 8, grp % 8
                        mm = tensor.matmul(
                            pwk(w, k),
                            cw_sb[ti][:, sblk * 128:(sblk + 1) * 128],
                            cw_sb[ti][:, T + hwg * 512: T + (hwg + 1) * 512],
                            start=(ti == 0), stop=(ti == 2),
                        )
                mm.then_inc(fold_sem, 1)
            # main
            off = 0
            last_cast_wait = 0
            for t, xg in enumerate(XTILES):
                tensor.wait_ge(x_sems[t], 16)
                for c in range(xg):
                    g = off + c
                    sblk, hw = g // 64, g % 64
                    grp = sblk * 8 + hw // 8
                    if grp + 1 > last_cast_wait:
                        last_cast_wait = grp + 1
                        tensor.wait_ge(cast_sem, last_cast_wait)
                    mm = tensor.matmul(
                        acc[:],
                        weff[:, sblk * 4096 + hw * 64: sblk * 4096 + (hw + 1) * 64],
                        xt_sb[t][:, c, :],
                        start=(g == 0), stop=(g == G - 1),
                    )
                off += xg
            mm.then_inc(acc_sem, 1)

        @block.vector
        def _(vector):
            for w in range(NWAVE):
                vector.wait_ge(fold_sem, w + 1)
                for k in range(WGRP):
                    grp = w * WGRP + k
                    sblk, hwg = grp // 8, grp % 8
                    dst = weff[:, sblk * 4096 + hwg * 512:
                               sblk * 4096 + (hwg + 1) * 512]
                    vector.tensor_copy(dst, pwk(w, k)).then_inc(cast_sem, 1)
            vector.wait_ge(acc_sem, 1)
            vector.wait_ge(bias_sem, 16)
            vector.tensor_scalar_add(t1[:], acc[:], bias[:]).then_inc(epi_sem, 1)
            vector.wait_ge(epi_sem, 1)
            vector.scalar_tensor_tensor(
                y[:], t1[:], NEG_SLOPE, t1[:],
                op0=mybir.AluOpType.mult, op1=mybir.AluOpType.max,
            ).then_inc(y_sem, 1)


def _emit_raw_hostfold(nc, xt, wf, bi, outT):
    """Host-folded variant: Weff arrives precomputed (weight preprocessing);
    the device runs the 2.1 GFLOP data contraction + epilogue."""
    f32 = mybir.dt.float32
    bf16 = mybir.dt.bfloat16

    weff = nc.alloc_sbuf_tensor("weff", [128, 2 * 64 * F], bf16).ap()
    xt_sb = [nc.alloc_sbuf_tensor(f"xs{t}", [128, xg, BC], bf16).ap()
             for t, xg in enumerate(XTILES)]
    wsrc = nc.alloc_sbuf_tensor("wsrc", [128, BC], bf16).ap()
    bias = nc.alloc_sbuf_tensor("bias", [F, 1], f32).ap()
    t1 = nc.alloc_sbuf_tensor("t1", [F, BC], f32).ap()
    y = nc.alloc_sbuf_tensor("y", [F, BC], f32).ap()

    with ExitStack() as es:
        acc = es.enter_context(nc.psum_tensor("accps", [F, BC], f32)).ap()
        wf_sems = [es.enter_context(nc.semaphore(f"wf{i}_sem")) for i in range(2)]
        x_sems = [es.enter_context(nc.semaphore(f"x{t}_sem"))
                  for t in range(len(XTILES))]
        bias_sem = es.enter_context(nc.semaphore("bias_sem"))
        out_sem = es.enter_context(nc.semaphore("out_sem"))
        ws_sem = es.enter_context(nc.semaphore("ws_sem"))
        acc_sem = es.enter_context(nc.semaphore("acc_sem"))
        epi_sem = es.enter_context(nc.semaphore("epi_sem"))
        y_sem = es.enter_context(nc.semaphore("y_sem"))
        block = es.enter_context(nc.Block(no_gpsimd_drain=True))

        @block.sync
        def _(sync):
            # weff half 0 (covers the first 64 chunks), first two x tiles,
            # then weff half 1, then the x stream
            sync.dma_start(weff[:, 0:4096], wf[:, 0:4096]).then_inc(wf_sems[0], 16)
            offs = np.cumsum([0] + XTILES)

            def xdma(t):
                foff = 128 * BC * int(offs[t])
                src = xt[foff: foff + 128 * XTILES[t] * BC].rearrange(
                    "(p c b) -> p c b", p=128, c=XTILES[t])
                sync.dma_start(xt_sb[t][:], src).then_inc(x_sems[t], 16)

            xdma(0)
            xdma(1)
            sync.dma_start(weff[:, 4096:], wf[:, 4096:]).then_inc(wf_sems[1], 16)
            for t in range(2, len(XTILES)):
                xdma(t)
            sync.wait_ge(y_sem, 1)
            sync.dma_start(outT[:], y[:]).then_inc(out_sem, 16)
            sync.wait_ge(out_sem, 16)

        @block.scalar
        def _(scalar):
            # tiny bias load on the otherwise idle second HWDGE ring, so it
            # never queues behind the 10 MB x stream
            scalar.dma_start(bias[:], bi[:]).then_inc(bias_sem, 16)

        @block.gpsimd
        def _(gpsimd):
            gpsimd.memset(wsrc[:], 0.0).then_inc(ws_sem, 1)

        @block.tensor
        def _(tensor):
            tensor.wait_ge(ws_sem, 1)
            for _ in range(NWARM):
                tensor.matmul(acc[0:1, :], wsrc[:, 0:1], wsrc[:],
                              start=True, stop=True)
            off = 0
            waited_wf1 = False
            for t, xg in enumerate(XTILES):
                tensor.wait_ge(x_sems[t], 16)
                if t == 0:
                    tensor.wait_ge(wf_sems[0], 16)
                for c in range(xg):
                    g = off + c
                    sblk, hw = g // 64, g % 64
                    if sblk == 1 and not waited_wf1:
                        waited_wf1 = True
                        tensor.wait_ge(wf_sems[1], 16)
                    mm = tensor.matmul(
                        acc[:],
                        weff[:, sblk * 4096 + hw * 64: sblk * 4096 + (hw + 1) * 64],
                        xt_sb[t][:, c, :],
                        start=(g == 0), stop=(g == G - 1),
                    )
                off += xg
            mm.then_inc(acc_sem, 1)

        @block.vector
        def _(vector):
            vector.wait_ge(acc_sem, 1)
            vector.wait_ge(bias_sem, 16)
            vector.tensor_scalar_add(t1[:], acc[:], bias[:]).then_inc(epi_sem, 1)
            vector.wait_ge(epi_sem, 1)
            vector.scalar_tensor_tensor(
                y[:], t1[:], NEG_SLOPE, t1[:],
                op0=mybir.AluOpType.mult, op1=mybir.AluOpType.max,
            ).then_inc(y_sem, 1)


_CACHE = {}


HOST_FOLD = True


def _get_kernel():
    if "nc" not in _CACHE:
        f32 = mybir.dt.float32
        bf16 = mybir.dt.bfloat16
        nc = bass.Bass("TRN2", target_bir_lowering=False, debug=False,
                      enable_partition_id=False)
        xt_d = nc.dram_tensor("xt", [G * 128 * BC], bf16, kind="ExternalInput")
        bi_d = nc.dram_tensor("bi", [F, 1], f32, kind="ExternalInput")
        out_d = nc.dram_tensor("outT", [F, BC], f32, kind="ExternalOutput")
        if HOST_FOLD:
            wf_d = nc.dram_tensor("wf", [128, 2 * 64 * F], bf16,
                                  kind="ExternalInput")
            _emit_raw_hostfold(nc, xt_d.ap(), wf_d.ap(), bi_d.ap(), out_d.ap())
        else:
            wt_d = nc.dram_tensor("wt", [TDWT, T + 64 * F], bf16,
                                  kind="ExternalInput")
            _emit_raw(nc, xt_d.ap(), wt_d.ap(), bi_d.ap(), out_d.ap())
        _CACHE["nc"] = nc
    return _CACHE["nc"]


def make_in_maps(x, W, b):
    import ml_dtypes
    bf16 = ml_dtypes.bfloat16
    dwt_m = _build_dwt_matrix()
    bi = np.ascontiguousarray(b.reshape(F, 1)).astype(np.float32)
    if HOST_FOLD:
        # weight preprocessing: fold the DWT matrix into the conv weight
        A = W[:, 0].reshape(F, TDWT, 64).transpose(1, 2, 0).reshape(TDWT, -1)
        weff = (dwt_m @ A.astype(np.float64)).reshape(T, 64, F)    # (s, hw, f)
        wf = np.ascontiguousarray(
            weff.reshape(2, 128, 64 * F).transpose(1, 0, 2)
        ).reshape(128, 2 * 64 * F).astype(bf16)
        wblob = {"wf": wf}
    else:
        dm = dwt_m.T                                               # (276, 256)
        wtc = W[:, 0].reshape(F, TDWT, 64).transpose(1, 2, 0).reshape(TDWT, -1)
        wt = np.ascontiguousarray(
            np.concatenate([dm, wtc], axis=1)).astype(bf16)        # (276, 4352)
        wblob = {"wt": wt}
    in_maps = []
    for c in range(NCORES):
        # chunk g = sblk*64 + hw holds rows [s_in, b]; tiles of XTILES[t]
        # chunks are stored back-to-back as [p, chunk, b] blocks so each
        # tile is one contiguous DMA.
        xc = x[c * BC:(c + 1) * BC, 0].astype(bf16)                # (BC, 256, 8, 8)
        xg = xc.reshape(BC, 2, 128, 64).transpose(1, 3, 2, 0)      # (sblk, hw, s_in, b)
        xg = xg.reshape(G, 128, BC)                                # (g, p, b)
        parts, off = [], 0
        for n in XTILES:
            parts.append(np.ascontiguousarray(
                xg[off:off + n].transpose(1, 0, 2)).reshape(-1))   # (p, c, b) flat
            off += n
        in_maps.append({"xt": np.concatenate(parts), "bi": bi, **wblob})
    return in_maps


def kernel(x, W, b, _trace=False):
    nc = _get_kernel()
    in_maps = make_in_maps(np.asarray(x), np.asarray(W), np.asarray(b))
    res = run_bass_kernel_spmd(nc, in_maps, list(range(NCORES)), trace=_trace)
    out = np.empty((B, F), np.float32)
    for c in range(NCORES):
        out[c * BC:(c + 1) * BC] = res.results[c]["outT"].T
    if _trace:
        return out, res
    return out
